# revision 7
# baseline (speedup 1.0000x reference)
"""MoE routed dynamics kernel for Trainium2 (8 NeuronCores, expert-parallel).

Problem: for each row b of a [B, D+A] input, route through one of P=8
two-layer MLPs selected by policy_indices[b]:
    h = relu(x @ W1[p] + b1[p]);  y = h @ W2[p] + b2[p]

Sharding: expert-parallel. Core p owns expert p's weights (resident in
SBUF) and processes exactly the rows routed to expert p. The all-to-all
dispatch keyed on policy_indices happens on the host at shard time
(gather rows by expert, pad to a common capacity C), and the inverse
scatter happens at unshard time.

Device kernel (per core), all activations kept feature-on-partition so
no transposes are needed anywhere:
    xT   [DA, C]  (DA=576)         input, transposed on host
    hT   [H, C]   = relu(W1.T @ x + b1), H=1024, via PE matmuls
    outT [D, C]   = W2.T @ h + b2,  D=512
Matmuls run as out[M,N] = lhsT.T @ rhs with lhsT = weight chunks in
their natural [K, M] layout and rhs = activation chunks [K, N<=512].

Matmul dtype is bfloat16 (host pre-casts): 1 PE cycle/row streaming and
half the DMA bytes of fp32. PSUM accumulation stays fp32, so the result
error vs the fp32 reference is ~1e-3 — far under the 2e-2 gate.

DMA queueing: each engine's DGE queue is in-order and is occupied for
the full transfer, so loads and stores must not share a queue (a store
that waits on compute would block the next chunk's prefetch). Layout is
packed host-side so each logical transfer is one descriptor:
  sync   queue: x loads (chunk 0 split per k-chunk for fast start)
  gpsimd queue: w1/b1/b2/w2 at start, then the output stores
  vector engine: bias-adds only
"""

import math

import numpy as np

_B = 16384
_P = 8
_D = 512
_A = 64
_H = 1024
_DA = _D + _A   # 576
_KC1 = 5        # k-chunks of layer 1: DA zero-padded to 5*128
_DAP = _KC1 * 128
_KC2 = _H // 128  # 8 k-chunks of layer 2
_MH = _H // 128   # 8 output tiles of layer 1
_MD = _D // 128   # 4 output tiles of layer 2
_N_CORES = 8

_kernel_cache: dict = {}


def _n_chunks(C: int):
    assert C % 128 == 0, C
    out = []
    n0 = 0
    while C - n0 > 512:
        out.append((n0, 512))
        n0 += 512
    out.append((n0, C - n0))
    return out


def _build_bass(C: int):
    import concourse.bacc as bacc
    import concourse.mybir as mybir
    from concourse.tile import TileContext

    fp32 = mybir.dt.float32
    bf16 = mybir.dt.bfloat16
    act = mybir.ActivationFunctionType

    chunks = _n_chunks(C)

    nc = bacc.Bacc()
    # xP: per chunk, the 5 k-chunk tiles stored contiguously [128, 5*nl].
    xP = nc.declare_dram_parameter("xP", [128, _KC1 * C], bf16, isOutput=False)
    # w1P: per m-group, its 5 [128,128] k-slices contiguous.
    w1 = nc.declare_dram_parameter("w1", [128, _MH * _KC1 * 128], bf16, isOutput=False)
    b1 = nc.declare_dram_parameter("b1", [128, _MH], fp32, isOutput=False)
    # w2P: per d-group, its 8 [128,128] k-slices contiguous.
    w2 = nc.declare_dram_parameter("w2", [128, _MD * _KC2 * 128], bf16, isOutput=False)
    b2 = nc.declare_dram_parameter("b2", [128, _MD], fp32, isOutput=False)
    outT = nc.declare_dram_parameter("outT", [_D, C], fp32, isOutput=True)

    with TileContext(nc) as tc:
        with (
            tc.tile_pool(name="wpool", bufs=1) as wpool,
            tc.tile_pool(name="xpool", bufs=3) as xpool,
            tc.tile_pool(name="hpool", bufs=2) as hpool,
            tc.tile_pool(name="ypool", bufs=6) as ypool,
            tc.tile_pool(name="ps1", bufs=4, space="PSUM") as ps1,
            tc.tile_pool(name="ps2", bufs=4, space="PSUM") as ps2,
        ):
            # Weights on the gpsimd queue in consumption order; m=0's k=0
            # slice goes alone first so the first LDWEIGHTS is gated by a
            # 32KB transfer, not a 160KB one. Biases ride the idle vector
            # queue so they don't delay w1 m=1 (which gates the second
            # matmul group).
            w1_sb = wpool.tile([128, _MH * _KC1 * 128], bf16, tag="w1")

            def w1_slab(m):
                return slice(m * _KC1 * 128, (m + 1) * _KC1 * 128)

            nc.gpsimd.dma_start(out=w1_sb[:, :128], in_=w1[:, :128])
            nc.gpsimd.dma_start(
                out=w1_sb[:, 128 : _KC1 * 128], in_=w1[:, 128 : _KC1 * 128]
            )
            b1_sb = wpool.tile([128, _MH], fp32, tag="b1")
            nc.scalar.dma_start(out=b1_sb[:], in_=b1[:, :])
            b2_sb = wpool.tile([128, _MD], fp32, tag="b2")
            nc.scalar.dma_start(out=b2_sb[:], in_=b2[:, :])
            for m in range(1, _MH):
                nc.gpsimd.dma_start(out=w1_sb[:, w1_slab(m)], in_=w1[:, w1_slab(m)])
            w2_sb = wpool.tile([128, _MD * _KC2 * 128], bf16, tag="w2")

            def w2_slab(d):
                return slice(d * _KC2 * 128, (d + 1) * _KC2 * 128)

            for d in range(_MD):
                nc.gpsimd.dma_start(out=w2_sb[:, w2_slab(d)], in_=w2[:, w2_slab(d)])

            # x loads on the sync queue. Chunk 0 split per k-chunk so the
            # first matmul starts after a small transfer; later chunks are
            # one descriptor each. Issued all up front: queue order ==
            # consumption order, buffer reuse gated by the pool semaphore.
            x_sb = []
            for ci, (n0, nl) in enumerate(chunks):
                xt = xpool.tile([128, _KC1 * nl], bf16, tag=f"x_{nl}")
                base = _KC1 * n0
                if ci == 0:
                    for i in range(_KC1):
                        nc.sync.dma_start(
                            out=xt[:, i * nl : (i + 1) * nl],
                            in_=xP[:, base + i * nl : base + (i + 1) * nl],
                        )
                else:
                    nc.sync.dma_start(
                        out=xt[:], in_=xP[:, base : base + _KC1 * nl]
                    )
                x_sb.append(xt)

            for ci, (n0, nl) in enumerate(chunks):
                xt = x_sb[ci]
                h_sb = []
                for m in range(_MH):
                    ps = ps1.tile([128, nl], fp32, tag="ps1")
                    for i in range(_KC1):
                        nc.tensor.matmul(
                            ps[:, :],
                            w1_sb[:, (m * _KC1 + i) * 128 : (m * _KC1 + i + 1) * 128],
                            xt[:, i * nl : (i + 1) * nl],
                            start=(i == 0),
                            stop=(i == _KC1 - 1),
                        )
                    ht = hpool.tile([128, nl], bf16, tag=f"h_{m}")
                    nc.scalar.activation(ht[:], ps[:], act.Relu, bias=b1_sb[:, m : m + 1])
                    h_sb.append(ht)

                for d in range(_MD):
                    ps = ps2.tile([128, nl], fp32, tag="ps2")
                    for m in range(_MH):
                        nc.tensor.matmul(
                            ps[:, :],
                            w2_sb[:, (d * _KC2 + m) * 128 : (d * _KC2 + m + 1) * 128],
                            h_sb[m][:, :],
                            start=(m == 0),
                            stop=(m == _MH - 1),
                        )
                    yt = ypool.tile([128, nl], fp32, tag="y")
                    nc.vector.tensor_scalar_add(yt[:], ps[:], b2_sb[:, d : d + 1])
                    # Stores alternate between the gpsimd and sync queues
                    # (both idle once weights/x are in); the final chunk's
                    # four stores fan out across four queues so the kernel
                    # tail is one store's latency, not four serialized.
                    if ci == len(chunks) - 1:
                        eng = [nc.gpsimd, nc.sync, nc.scalar, nc.gpsimd][d]
                    else:
                        eng = nc.gpsimd if (ci * _MD + d) % 2 == 0 else nc.sync
                    eng.dma_start(
                        out=outT[d * 128 : (d + 1) * 128, n0 : n0 + nl],
                        in_=yt[:],
                    )

    nc.compile()
    return nc


def _get_bass(C: int):
    nc = _kernel_cache.get(C)
    if nc is None:
        nc = _build_bass(C)
        _kernel_cache[C] = nc
    return nc


def _prepare_in_maps(latents, actions, policy_indices, W1, b1, W2, b2):
    """Expert-parallel dispatch: returns (in_maps, C, order, offs, counts)."""
    import ml_dtypes

    bf16 = ml_dtypes.bfloat16

    latents = np.asarray(latents, dtype=np.float32)
    actions = np.asarray(actions, dtype=np.float32)
    pi = np.asarray(policy_indices).astype(np.int64)
    W1 = np.asarray(W1, dtype=np.float32)
    b1 = np.asarray(b1, dtype=np.float32)
    W2 = np.asarray(W2, dtype=np.float32)
    b2 = np.asarray(b2, dtype=np.float32)

    B = latents.shape[0]
    counts = np.bincount(pi, minlength=_P)
    order = np.argsort(pi, kind="stable")
    offs = np.concatenate(([0], np.cumsum(counts)))

    C = max(256, int(math.ceil(counts.max() / 128)) * 128)
    chunks = _n_chunks(C)

    x = np.empty((B, _DA), dtype=np.float32)
    x[:, :_D] = latents
    x[:, _D:] = actions
    x_sorted = x[order]

    in_maps = []
    for p in range(_P):
        xp = np.zeros((_DAP, C), dtype=bf16)
        xp[:_DA, : counts[p]] = x_sorted[offs[p] : offs[p + 1]].T.astype(bf16)
        x3 = xp.reshape(_KC1, 128, C)
        xP = np.concatenate(
            [
                x3[:, :, n0 : n0 + nl].transpose(1, 0, 2).reshape(128, _KC1 * nl)
                for (n0, nl) in chunks
            ],
            axis=1,
        )
        w1p = np.zeros((_DAP, _H), dtype=bf16)
        w1p[:_DA] = W1[p].astype(bf16)
        # [5,128,8,128] -> [128, m, k, 128]
        w1P = np.ascontiguousarray(
            w1p.reshape(_KC1, 128, _MH, 128).transpose(1, 2, 0, 3).reshape(128, -1)
        )
        w2P = np.ascontiguousarray(
            W2[p].astype(bf16).reshape(_KC2, 128, _MD, 128).transpose(1, 2, 0, 3).reshape(128, -1)
        )
        in_maps.append(
            {
                "xP": np.ascontiguousarray(xP),
                "w1": w1P,
                "b1": np.ascontiguousarray(b1[p].reshape(_MH, 128).T),
                "w2": w2P,
                "b2": np.ascontiguousarray(b2[p].reshape(_MD, 128).T),
            }
        )
    return in_maps, C, order, offs, counts


def kernel(latents, actions, policy_indices, W1, b1, W2, b2):
    from concourse.bass_utils import run_bass_kernel_spmd

    in_maps, C, order, offs, counts = _prepare_in_maps(
        latents, actions, policy_indices, W1, b1, W2, b2
    )
    nc = _get_bass(C)
    results = run_bass_kernel_spmd(nc, in_maps, list(range(_N_CORES))).results

    B = np.asarray(latents).shape[0]
    out = np.empty((B, _D), dtype=np.float32)
    for p in range(_P):
        yT = results[p]["outT"]
        out[order[offs[p] : offs[p + 1]]] = yT[:, : counts[p]].T
    return out


# revision 9
# speedup vs baseline: 1.1228x; 1.1228x over previous
"""MoE routed dynamics kernel for Trainium2 (8 NeuronCores, expert-parallel).

Problem: for each row b of a [B, D+A] input, route through one of P=8
two-layer MLPs selected by policy_indices[b]:
    h = relu(x @ W1[p] + b1[p]);  y = h @ W2[p] + b2[p]

Sharding: expert-parallel. Core p owns expert p's weights (resident in
SBUF) and processes exactly the rows routed to expert p. The all-to-all
dispatch keyed on policy_indices happens on the host at shard time
(gather rows by expert, pad to a common capacity C), and the inverse
scatter happens at unshard time.

Device kernel (per core), all activations kept feature-on-partition so
no transposes are needed anywhere:
    xT   [DA, C]  (DA=576)         input, transposed on host
    hT   [H, C]   = relu(W1.T @ x + b1), H=1024, via PE matmuls
    outT [D, C]   = W2.T @ h + b2,  D=512
Matmuls run as out[M,N] = lhsT.T @ rhs with lhsT = weight chunks in
their natural [K, M] layout and rhs = activation chunks [K, N<=512].

Matmul dtype is bfloat16 (host pre-casts): 1 PE cycle/row streaming and
half the DMA bytes of fp32. PSUM accumulation stays fp32, so the result
error vs the fp32 reference is ~1e-3 — far under the 2e-2 gate.

DMA queueing: each engine's DGE queue is in-order and is occupied for
the full transfer, so loads and stores must not share a queue (a store
that waits on compute would block the next chunk's prefetch). Layout is
packed host-side so each logical transfer is one descriptor:
  sync   queue: x loads (chunk 0 split per k-chunk for fast start)
  gpsimd queue: w1/b1/b2/w2 at start, then the output stores
  vector engine: bias-adds only
"""

import math

import numpy as np

_B = 16384
_P = 8
_D = 512
_A = 64
_H = 1024
_DA = _D + _A   # 576
_KC1 = 5        # k-chunks of layer 1: DA zero-padded to 5*128
_DAP = _KC1 * 128
_KC2 = _H // 128  # 8 k-chunks of layer 2
_MH = _H // 128   # 8 output tiles of layer 1
_MD = _D // 128   # 4 output tiles of layer 2
_N_CORES = 8

_kernel_cache: dict = {}


def _n_chunks(C: int):
    assert C % 128 == 0, C
    out = []
    n0 = 0
    while C - n0 > 512:
        out.append((n0, 512))
        n0 += 512
    out.append((n0, C - n0))
    return out


def _build_bass(C: int):
    import concourse.bacc as bacc
    import concourse.mybir as mybir
    from concourse.tile import TileContext

    fp32 = mybir.dt.float32
    bf16 = mybir.dt.bfloat16
    act = mybir.ActivationFunctionType

    chunks = _n_chunks(C)

    nc = bacc.Bacc()
    # xP: per chunk, the 5 k-chunk tiles stored contiguously [128, 5*nl].
    xP = nc.declare_dram_parameter("xP", [128, _KC1 * C], bf16, isOutput=False)
    # w1P: per m-group, its 5 [128,128] k-slices contiguous.
    w1 = nc.declare_dram_parameter("w1", [128, _MH * _KC1 * 128], bf16, isOutput=False)
    b1 = nc.declare_dram_parameter("b1", [128, _MH], fp32, isOutput=False)
    # w2P: per d-group, its 8 [128,128] k-slices contiguous.
    w2 = nc.declare_dram_parameter("w2", [128, _MD * _KC2 * 128], bf16, isOutput=False)
    b2 = nc.declare_dram_parameter("b2", [128, _MD], fp32, isOutput=False)
    outT = nc.declare_dram_parameter("outT", [_D, C], fp32, isOutput=True)

    with TileContext(nc) as tc:
        with (
            tc.tile_pool(name="wpool", bufs=1) as wpool,
            tc.tile_pool(name="xpool", bufs=3) as xpool,
            tc.tile_pool(name="hpool", bufs=2) as hpool,
            tc.tile_pool(name="ypool", bufs=6) as ypool,
            tc.tile_pool(name="ps1", bufs=4, space="PSUM") as ps1,
            tc.tile_pool(name="ps2", bufs=4, space="PSUM") as ps2,
        ):
            # Weights on the gpsimd queue in consumption order; m=0's k=0
            # slice goes alone first so the first LDWEIGHTS is gated by a
            # 32KB transfer, not a 160KB one. Biases ride the idle vector
            # queue so they don't delay w1 m=1 (which gates the second
            # matmul group).
            w1_sb = wpool.tile([128, _MH * _KC1 * 128], bf16, tag="w1")

            def w1_slab(m):
                return slice(m * _KC1 * 128, (m + 1) * _KC1 * 128)

            nc.gpsimd.dma_start(out=w1_sb[:, :128], in_=w1[:, :128])
            nc.gpsimd.dma_start(
                out=w1_sb[:, 128 : _KC1 * 128], in_=w1[:, 128 : _KC1 * 128]
            )
            nc.gpsimd.dma_start(out=w1_sb[:, w1_slab(1)], in_=w1[:, w1_slab(1)])
            b1_sb = wpool.tile([128, _MH], fp32, tag="b1")
            nc.gpsimd.dma_start(out=b1_sb[:], in_=b1[:, :])
            b2_sb = wpool.tile([128, _MD], fp32, tag="b2")
            nc.gpsimd.dma_start(out=b2_sb[:], in_=b2[:, :])
            for m in range(2, _MH):
                nc.gpsimd.dma_start(out=w1_sb[:, w1_slab(m)], in_=w1[:, w1_slab(m)])
            w2_sb = wpool.tile([128, _MD * _KC2 * 128], bf16, tag="w2")

            def w2_slab(d):
                return slice(d * _KC2 * 128, (d + 1) * _KC2 * 128)

            for d in range(_MD):
                nc.gpsimd.dma_start(out=w2_sb[:, w2_slab(d)], in_=w2[:, w2_slab(d)])

            # x loads on the sync queue. Chunk 0 split per k-chunk so the
            # first matmul starts after a small transfer; later chunks are
            # one descriptor each. Issued all up front: queue order ==
            # consumption order, buffer reuse gated by the pool semaphore.
            x_sb = []
            for ci, (n0, nl) in enumerate(chunks):
                xt = xpool.tile([128, _KC1 * nl], bf16, tag=f"x_{nl}")
                base = _KC1 * n0
                if ci == 0:
                    for i in range(_KC1):
                        nc.sync.dma_start(
                            out=xt[:, i * nl : (i + 1) * nl],
                            in_=xP[:, base + i * nl : base + (i + 1) * nl],
                        )
                else:
                    nc.sync.dma_start(
                        out=xt[:], in_=xP[:, base : base + _KC1 * nl]
                    )
                x_sb.append(xt)

            for ci, (n0, nl) in enumerate(chunks):
                xt = x_sb[ci]
                h_sb = []
                for m in range(_MH):
                    ps = ps1.tile([128, nl], fp32, tag="ps1")
                    for i in range(_KC1):
                        nc.tensor.matmul(
                            ps[:, :],
                            w1_sb[:, (m * _KC1 + i) * 128 : (m * _KC1 + i + 1) * 128],
                            xt[:, i * nl : (i + 1) * nl],
                            start=(i == 0),
                            stop=(i == _KC1 - 1),
                        )
                    ht = hpool.tile([128, nl], bf16, tag=f"h_{m}")
                    nc.scalar.activation(ht[:], ps[:], act.Relu, bias=b1_sb[:, m : m + 1])
                    h_sb.append(ht)

                for d in range(_MD):
                    ps = ps2.tile([128, nl], fp32, tag="ps2")
                    for m in range(_MH):
                        nc.tensor.matmul(
                            ps[:, :],
                            w2_sb[:, (d * _KC2 + m) * 128 : (d * _KC2 + m + 1) * 128],
                            h_sb[m][:, :],
                            start=(m == 0),
                            stop=(m == _MH - 1),
                        )
                    yt = ypool.tile([128, nl], fp32, tag="y")
                    nc.vector.tensor_scalar_add(yt[:], ps[:], b2_sb[:, d : d + 1])
                    # Stores alternate between the gpsimd and sync queues
                    # (both idle once weights/x are in); the final chunk's
                    # four stores fan out across four queues so the kernel
                    # tail is one store's latency, not four serialized.
                    if ci == len(chunks) - 1:
                        eng = [nc.gpsimd, nc.sync, nc.scalar, nc.gpsimd][d]
                    else:
                        eng = nc.gpsimd
                    eng.dma_start(
                        out=outT[d * 128 : (d + 1) * 128, n0 : n0 + nl],
                        in_=yt[:],
                    )

    nc.compile()
    return nc


def _get_bass(C: int):
    nc = _kernel_cache.get(C)
    if nc is None:
        nc = _build_bass(C)
        _kernel_cache[C] = nc
    return nc


def _prepare_in_maps(latents, actions, policy_indices, W1, b1, W2, b2):
    """Expert-parallel dispatch: returns (in_maps, C, order, offs, counts)."""
    import ml_dtypes

    bf16 = ml_dtypes.bfloat16

    latents = np.asarray(latents, dtype=np.float32)
    actions = np.asarray(actions, dtype=np.float32)
    pi = np.asarray(policy_indices).astype(np.int64)
    W1 = np.asarray(W1, dtype=np.float32)
    b1 = np.asarray(b1, dtype=np.float32)
    W2 = np.asarray(W2, dtype=np.float32)
    b2 = np.asarray(b2, dtype=np.float32)

    B = latents.shape[0]
    counts = np.bincount(pi, minlength=_P)
    order = np.argsort(pi, kind="stable")
    offs = np.concatenate(([0], np.cumsum(counts)))

    C = max(256, int(math.ceil(counts.max() / 128)) * 128)
    chunks = _n_chunks(C)

    x = np.empty((B, _DA), dtype=np.float32)
    x[:, :_D] = latents
    x[:, _D:] = actions
    x_sorted = x[order]

    in_maps = []
    for p in range(_P):
        xp = np.zeros((_DAP, C), dtype=bf16)
        xp[:_DA, : counts[p]] = x_sorted[offs[p] : offs[p + 1]].T.astype(bf16)
        x3 = xp.reshape(_KC1, 128, C)
        xP = np.concatenate(
            [
                x3[:, :, n0 : n0 + nl].transpose(1, 0, 2).reshape(128, _KC1 * nl)
                for (n0, nl) in chunks
            ],
            axis=1,
        )
        w1p = np.zeros((_DAP, _H), dtype=bf16)
        w1p[:_DA] = W1[p].astype(bf16)
        # [5,128,8,128] -> [128, m, k, 128]
        w1P = np.ascontiguousarray(
            w1p.reshape(_KC1, 128, _MH, 128).transpose(1, 2, 0, 3).reshape(128, -1)
        )
        w2P = np.ascontiguousarray(
            W2[p].astype(bf16).reshape(_KC2, 128, _MD, 128).transpose(1, 2, 0, 3).reshape(128, -1)
        )
        in_maps.append(
            {
                "xP": np.ascontiguousarray(xP),
                "w1": w1P,
                "b1": np.ascontiguousarray(b1[p].reshape(_MH, 128).T),
                "w2": w2P,
                "b2": np.ascontiguousarray(b2[p].reshape(_MD, 128).T),
            }
        )
    return in_maps, C, order, offs, counts


def kernel(latents, actions, policy_indices, W1, b1, W2, b2):
    from concourse.bass_utils import run_bass_kernel_spmd

    in_maps, C, order, offs, counts = _prepare_in_maps(
        latents, actions, policy_indices, W1, b1, W2, b2
    )
    nc = _get_bass(C)
    results = run_bass_kernel_spmd(nc, in_maps, list(range(_N_CORES))).results

    B = np.asarray(latents).shape[0]
    out = np.empty((B, _D), dtype=np.float32)
    for p in range(_P):
        yT = results[p]["outT"]
        out[order[offs[p] : offs[p + 1]]] = yT[:, : counts[p]].T
    return out


# revision 10
# speedup vs baseline: 1.1697x; 1.0417x over previous
"""MoE routed dynamics kernel for Trainium2 (8 NeuronCores, expert-parallel).

Problem: for each row b of a [B, D+A] input, route through one of P=8
two-layer MLPs selected by policy_indices[b]:
    h = relu(x @ W1[p] + b1[p]);  y = h @ W2[p] + b2[p]

Sharding: expert-parallel. Core p owns expert p's weights (resident in
SBUF) and processes exactly the rows routed to expert p. The all-to-all
dispatch keyed on policy_indices happens on the host at shard time
(gather rows by expert, pad to a common capacity C), and the inverse
scatter happens at unshard time.

Device kernel (per core), all activations kept feature-on-partition so
no transposes are needed anywhere:
    xT   [DA, C]  (DA=576)         input, transposed on host
    hT   [H, C]   = relu(W1.T @ x + b1), H=1024, via PE matmuls
    outT [D, C]   = W2.T @ h + b2,  D=512
Matmuls run as out[M,N] = lhsT.T @ rhs with lhsT = weight chunks in
their natural [K, M] layout and rhs = activation chunks [K, N<=512].

Matmul dtype is bfloat16 (host pre-casts): 1 PE cycle/row streaming and
half the DMA bytes of fp32. PSUM accumulation stays fp32, so the result
error vs the fp32 reference is ~1e-3 — far under the 2e-2 gate.

DMA queueing: each engine's DGE queue is in-order and is occupied for
the full transfer, so loads and stores must not share a queue (a store
that waits on compute would block the next chunk's prefetch). Layout is
packed host-side so each logical transfer is one descriptor:
  sync   queue: x loads (chunk 0 split per k-chunk for fast start)
  gpsimd queue: w1/b1/b2/w2 at start, then the output stores
  vector engine: bias-adds only
"""

import math

import numpy as np

_B = 16384
_P = 8
_D = 512
_A = 64
_H = 1024
_DA = _D + _A   # 576
_KC1 = 5        # k-chunks of layer 1: DA zero-padded to 5*128
_DAP = _KC1 * 128
_KC2 = _H // 128  # 8 k-chunks of layer 2
_MH = _H // 128   # 8 output tiles of layer 1
_MD = _D // 128   # 4 output tiles of layer 2
_N_CORES = 8

_kernel_cache: dict = {}


def _n_chunks(C: int):
    assert C % 128 == 0, C
    out = []
    n0 = 0
    while C - n0 > 512:
        out.append((n0, 512))
        n0 += 512
    out.append((n0, C - n0))
    return out


def _build_bass(C: int):
    import concourse.bacc as bacc
    import concourse.mybir as mybir
    from concourse.tile import TileContext

    fp32 = mybir.dt.float32
    bf16 = mybir.dt.bfloat16
    act = mybir.ActivationFunctionType

    chunks = _n_chunks(C)

    nc = bacc.Bacc()
    # xP: per chunk, the 5 k-chunk tiles stored contiguously [128, 5*nl].
    xP = nc.declare_dram_parameter("xP", [128, _KC1 * C], bf16, isOutput=False)
    # w1P: per m-group, its 5 [128,128] k-slices contiguous.
    w1 = nc.declare_dram_parameter("w1", [128, _MH * _KC1 * 128], bf16, isOutput=False)
    b1 = nc.declare_dram_parameter("b1", [128, _MH], fp32, isOutput=False)
    # w2P: per d-group, its 8 [128,128] k-slices contiguous.
    w2 = nc.declare_dram_parameter("w2", [128, _MD * _KC2 * 128], bf16, isOutput=False)
    b2 = nc.declare_dram_parameter("b2", [128, _MD], fp32, isOutput=False)
    outT = nc.declare_dram_parameter("outT", [_D, C], fp32, isOutput=True)

    with TileContext(nc) as tc:
        with (
            tc.tile_pool(name="wpool", bufs=1) as wpool,
            tc.tile_pool(name="xpool", bufs=3) as xpool,
            tc.tile_pool(name="hpool", bufs=2) as hpool,
            tc.tile_pool(name="ypool", bufs=6) as ypool,
            tc.tile_pool(name="ps1", bufs=4, space="PSUM") as ps1,
            tc.tile_pool(name="ps2", bufs=4, space="PSUM") as ps2,
        ):
            # Weights on the gpsimd queue in consumption order; m=0's k=0
            # slice goes alone first so the first LDWEIGHTS is gated by a
            # 32KB transfer, not a 160KB one. Biases ride the idle vector
            # queue so they don't delay w1 m=1 (which gates the second
            # matmul group).
            w1_sb = wpool.tile([128, _MH * _KC1 * 128], bf16, tag="w1")

            def w1_slab(m):
                return slice(m * _KC1 * 128, (m + 1) * _KC1 * 128)

            nc.gpsimd.dma_start(out=w1_sb[:, :128], in_=w1[:, :128])
            nc.gpsimd.dma_start(
                out=w1_sb[:, 128 : _KC1 * 128], in_=w1[:, 128 : _KC1 * 128]
            )
            nc.gpsimd.dma_start(out=w1_sb[:, w1_slab(1)], in_=w1[:, w1_slab(1)])
            b1_sb = wpool.tile([128, _MH], fp32, tag="b1")
            nc.gpsimd.dma_start(out=b1_sb[:], in_=b1[:, :])
            b2_sb = wpool.tile([128, _MD], fp32, tag="b2")
            nc.gpsimd.dma_start(out=b2_sb[:], in_=b2[:, :])
            for m in range(2, _MH):
                nc.gpsimd.dma_start(out=w1_sb[:, w1_slab(m)], in_=w1[:, w1_slab(m)])
            w2_sb = wpool.tile([128, _MD * _KC2 * 128], bf16, tag="w2")

            def w2_slab(d):
                return slice(d * _KC2 * 128, (d + 1) * _KC2 * 128)

            # x loads on the sync queue. Chunk 0 split per k-chunk so the
            # first matmul starts after a small transfer; later chunks are
            # one descriptor each. w2 rides sync right after chunk 0 —
            # chunk 0's layer 2 starts ~17us in, and the gpsimd queue alone
            # can't stream w1+w2 by then. Issued all up front: queue order
            # == consumption order, buffer reuse gated by pool semaphores.
            x_sb = []
            for ci, (n0, nl) in enumerate(chunks):
                xt = xpool.tile([128, _KC1 * nl], bf16, tag=f"x_{nl}")
                base = _KC1 * n0
                if ci == 0:
                    for i in range(_KC1):
                        nc.sync.dma_start(
                            out=xt[:, i * nl : (i + 1) * nl],
                            in_=xP[:, base + i * nl : base + (i + 1) * nl],
                        )
                    for d in range(_MD):
                        nc.sync.dma_start(
                            out=w2_sb[:, w2_slab(d)], in_=w2[:, w2_slab(d)]
                        )
                else:
                    nc.sync.dma_start(
                        out=xt[:], in_=xP[:, base : base + _KC1 * nl]
                    )
                x_sb.append(xt)

            for ci, (n0, nl) in enumerate(chunks):
                xt = x_sb[ci]
                h_sb = []
                for m in range(_MH):
                    ps = ps1.tile([128, nl], fp32, tag="ps1")
                    for i in range(_KC1):
                        nc.tensor.matmul(
                            ps[:, :],
                            w1_sb[:, (m * _KC1 + i) * 128 : (m * _KC1 + i + 1) * 128],
                            xt[:, i * nl : (i + 1) * nl],
                            start=(i == 0),
                            stop=(i == _KC1 - 1),
                        )
                    ht = hpool.tile([128, nl], bf16, tag=f"h_{m}")
                    nc.scalar.activation(ht[:], ps[:], act.Relu, bias=b1_sb[:, m : m + 1])
                    h_sb.append(ht)

                for d in range(_MD):
                    ps = ps2.tile([128, nl], fp32, tag="ps2")
                    for m in range(_MH):
                        nc.tensor.matmul(
                            ps[:, :],
                            w2_sb[:, (d * _KC2 + m) * 128 : (d * _KC2 + m + 1) * 128],
                            h_sb[m][:, :],
                            start=(m == 0),
                            stop=(m == _MH - 1),
                        )
                    yt = ypool.tile([128, nl], fp32, tag="y")
                    nc.vector.tensor_scalar_add(yt[:], ps[:], b2_sb[:, d : d + 1])
                    # Stores alternate between the gpsimd and sync queues
                    # (both idle once weights/x are in); the final chunk's
                    # four stores fan out across four queues so the kernel
                    # tail is one store's latency, not four serialized.
                    if ci == len(chunks) - 1:
                        eng = [nc.gpsimd, nc.sync, nc.scalar, nc.gpsimd][d]
                    else:
                        eng = nc.gpsimd
                    eng.dma_start(
                        out=outT[d * 128 : (d + 1) * 128, n0 : n0 + nl],
                        in_=yt[:],
                    )

    nc.compile()
    return nc


def _get_bass(C: int):
    nc = _kernel_cache.get(C)
    if nc is None:
        nc = _build_bass(C)
        _kernel_cache[C] = nc
    return nc


def _prepare_in_maps(latents, actions, policy_indices, W1, b1, W2, b2):
    """Expert-parallel dispatch: returns (in_maps, C, order, offs, counts)."""
    import ml_dtypes

    bf16 = ml_dtypes.bfloat16

    latents = np.asarray(latents, dtype=np.float32)
    actions = np.asarray(actions, dtype=np.float32)
    pi = np.asarray(policy_indices).astype(np.int64)
    W1 = np.asarray(W1, dtype=np.float32)
    b1 = np.asarray(b1, dtype=np.float32)
    W2 = np.asarray(W2, dtype=np.float32)
    b2 = np.asarray(b2, dtype=np.float32)

    B = latents.shape[0]
    counts = np.bincount(pi, minlength=_P)
    order = np.argsort(pi, kind="stable")
    offs = np.concatenate(([0], np.cumsum(counts)))

    C = max(256, int(math.ceil(counts.max() / 128)) * 128)
    chunks = _n_chunks(C)

    x = np.empty((B, _DA), dtype=np.float32)
    x[:, :_D] = latents
    x[:, _D:] = actions
    x_sorted = x[order]

    in_maps = []
    for p in range(_P):
        xp = np.zeros((_DAP, C), dtype=bf16)
        xp[:_DA, : counts[p]] = x_sorted[offs[p] : offs[p + 1]].T.astype(bf16)
        x3 = xp.reshape(_KC1, 128, C)
        xP = np.concatenate(
            [
                x3[:, :, n0 : n0 + nl].transpose(1, 0, 2).reshape(128, _KC1 * nl)
                for (n0, nl) in chunks
            ],
            axis=1,
        )
        w1p = np.zeros((_DAP, _H), dtype=bf16)
        w1p[:_DA] = W1[p].astype(bf16)
        # [5,128,8,128] -> [128, m, k, 128]
        w1P = np.ascontiguousarray(
            w1p.reshape(_KC1, 128, _MH, 128).transpose(1, 2, 0, 3).reshape(128, -1)
        )
        w2P = np.ascontiguousarray(
            W2[p].astype(bf16).reshape(_KC2, 128, _MD, 128).transpose(1, 2, 0, 3).reshape(128, -1)
        )
        in_maps.append(
            {
                "xP": np.ascontiguousarray(xP),
                "w1": w1P,
                "b1": np.ascontiguousarray(b1[p].reshape(_MH, 128).T),
                "w2": w2P,
                "b2": np.ascontiguousarray(b2[p].reshape(_MD, 128).T),
            }
        )
    return in_maps, C, order, offs, counts


def kernel(latents, actions, policy_indices, W1, b1, W2, b2):
    from concourse.bass_utils import run_bass_kernel_spmd

    in_maps, C, order, offs, counts = _prepare_in_maps(
        latents, actions, policy_indices, W1, b1, W2, b2
    )
    nc = _get_bass(C)
    results = run_bass_kernel_spmd(nc, in_maps, list(range(_N_CORES))).results

    B = np.asarray(latents).shape[0]
    out = np.empty((B, _D), dtype=np.float32)
    for p in range(_P):
        yT = results[p]["outT"]
        out[order[offs[p] : offs[p + 1]]] = yT[:, : counts[p]].T
    return out
